# revision 33
# baseline (speedup 1.0000x reference)
"""AttnBlock (GroupNorm -> single-head self-attention -> residual) on 8 TRN2 cores.

Sharding: B=4 batch elements x 2 query-token halves = 8 cores (SPMD, no
collectives).  Each core receives the full (rolled) channel-major batch
element x^T [C=256, HW=4096], computes GroupNorm + k/v for all 4096
tokens, and q/scores/attention/out-proj for its 2048-token half.

Numerics plan (rel-err budget 2e-2; measured ~5e-3):
  x arrives twice: bf16 (GN stats/apply -- fast DMA) and f32 (residual).
  GN apply -> hs bf16.  q/k/v projections in bf16 (fp32 PSUM).
  qT/kT/v quantized to fp8e4m3; scores and attn@v run fp8 DoubleRow
  matmuls (K=256 per instruction via [128,2,*] APs).  exp(S/16 - 2)
  emitted as fp8 (the -2 cancels between numerator and denominator and
  keeps exp < 55 << 448 = e4m3 max).  Softmax division happens on the
  normalized o^T in bf16 before the bf16 out-projection.

Layout is channel-major throughout (tokens on the free axis):
  hs^T = GN(x^T)                      [C, N]   bf16
  q^T/k^T = W^T.T @ hs^T + b          [C, NQ/N] fp8
  v   = hs^T.T @ Wv^T + bv            [N, C]   fp8 (row-major)
  S^T = kT.T @ qT   (DoubleRow)       [N, SW]  per 512-wide query strip
  P^T = exp(S^T/16 - 2) -> fp8
  Z   = ones.T @ P^T (DoubleRow, M=1) [1, SW]
  o^T = (v.T @ P^T) * (1/Z)           [C, SW]  bf16
  out^T = Wo^T.T @ o^T; final = (x + out + bo) * 2^-0.5
"""

import numpy as np
import ml_dtypes

import concourse.bass as bass
import concourse.tile as tile
from concourse import bacc, mybir
from concourse.bass_utils import run_bass_kernel_spmd

dt = mybir.dt
F32, F32R, BF16, FP8 = dt.float32, dt.float32r, dt.bfloat16, dt.float8e4
AF = mybir.ActivationFunctionType
ALU = mybir.AluOpType
DR = mybir.MatmulPerfMode.DoubleRow

P = 128          # partitions
C = 256          # channels
N = 4096         # tokens per batch element (64*64)
NQ = 2048        # query tokens per core
SW = 512         # query strip width
NS = NQ // SW    # 4 strips
MT = N // P      # 32 key m-tiles
MP = MT // 2     # 16 key m-tile pairs (DoubleRow)
GS = 8           # channels per group (256 / 32 groups)
EPS = 1e-6
ISCALE = 1.0 / 16.0       # attention scale c**-0.5
EBIAS = -2.0              # exp range shift; cancels in softmax
RS2 = float(2.0 ** -0.5)  # output residual scale

_prog_cache = {}


def _build_nc():
    nc = bacc.Bacc("TRN2", target_bir_lowering=False, debug=False, num_devices=8)

    def inp(name, shape, d=F32):
        return nc.dram_tensor(name, shape, d, kind="ExternalInput").ap()

    xbf_d = inp("xbf", [2, P, N], BF16)    # [c_half, c_in, n] bf16 (GN path)
    xf_d = inp("xf", [2, P, N], F32)       # f32 copy (residual path)
    wq_d = inp("wq16", [2, P, C], BF16)    # [ko, ci_in, c_out] = Wq.T halves
    wk_d = inp("wk16", [2, P, C], BF16)
    wv_d = inp("wv16", [2, P, C], BF16)
    wo_d = inp("wo16", [2, P, C], BF16)
    bq_d = inp("bqp", [P, 2])              # [c_out_in, c_out_half]
    bk_d = inp("bkp", [P, 2])
    bos_d = inp("bosp", [P, 2])            # bo * 2^-0.5, packed
    bv_d = inp("bv", [1, C])
    gnw_d = inp("gnw", [P, 2])
    gnb_d = inp("gnb", [P, 2])
    amat_d = inp("amat", [P, P])           # block-diag 8x8 of 1/8
    onesb_d = inp("onesb", [P, P])         # all-ones (rows 0:4 used)
    ones8_d = inp("ones8", [P, 32], FP8)   # fp8 ones, 16B-strided pairs
    out_d = nc.dram_tensor("out", [2, P, NQ], F32, kind="ExternalOutput").ap()

    with tile.TileContext(nc) as tc:
        with (
            tc.tile_pool(name="singles", bufs=1) as singles,
            tc.tile_pool(name="xpool", bufs=1) as xpool,
            tc.tile_pool(name="hsp", bufs=1) as hsp,
            tc.tile_pool(name="qk", bufs=1) as qk,
            tc.tile_pool(name="vpool", bufs=1) as vpool,
            tc.tile_pool(name="espool", bufs=2) as espool,
            tc.tile_pool(name="opool", bufs=2) as opool,
            tc.tile_pool(name="zfpool", bufs=2) as zfpool,
            tc.tile_pool(name="finpool", bufs=2) as finpool,
            tc.tile_pool(name="small", bufs=2) as small,
            tc.tile_pool(name="ps", bufs=2, space="PSUM") as ps,      # 4 banks
            tc.tile_pool(name="pz", bufs=1, space="PSUM") as pz,      # 1 bank
            tc.tile_pool(name="po", bufs=3, space="PSUM") as po,      # 3 banks
        ):
            # ---- x loads: bf16 halves first (GN critical path), f32 after ----
            xb0 = xpool.tile([P, N], BF16, tag="xb0")
            xb1 = xpool.tile([P, N], BF16, tag="xb1")
            xf0 = xpool.tile([P, N], F32, tag="xf0")
            xf1 = xpool.tile([P, N], F32, tag="xf1")
            for h in range(4):
                sl = slice(h * 1024, (h + 1) * 1024)
                nc.sync.dma_start(xb0[:, sl], xbf_d[0, :, sl])
                nc.scalar.dma_start(xb1[:, sl], xbf_d[1, :, sl])
            xbs = (xb0, xb1)
            xfs = (xf0, xf1)

            # ---- constants / weights (gpsimd queue) ----
            wq = singles.tile([P, 2, C], BF16)
            wk = singles.tile([P, 2, C], BF16)
            wv = singles.tile([P, 2, C], BF16)
            wo = singles.tile([P, 2, C], BF16)
            for t_, d_ in ((wq, wq_d), (wk, wk_d), (wv, wv_d), (wo, wo_d)):
                for ko in range(2):
                    nc.gpsimd.dma_start(t_[:, ko, :], d_[ko])
            amat = singles.tile([P, P], F32R)
            nc.gpsimd.dma_start(amat[:], amat_d.bitcast(F32R))
            onesb = singles.tile([P, P], F32R)
            nc.gpsimd.dma_start(onesb[:], onesb_d.bitcast(F32R))
            ones8 = singles.tile([P, 2, 16], FP8)
            nc.gpsimd.dma_start(
                ones8[:].rearrange("p a b -> p (a b)"), ones8_d)
            bq = singles.tile([P, 2], F32)
            nc.gpsimd.dma_start(bq[:], bq_d)
            bk = singles.tile([P, 2], F32)
            nc.gpsimd.dma_start(bk[:], bk_d)
            bos = singles.tile([P, 2], F32)
            nc.gpsimd.dma_start(bos[:], bos_d)
            gnw = singles.tile([P, 2], F32)
            nc.gpsimd.dma_start(gnw[:], gnw_d)
            gnb = singles.tile([P, 2], F32)
            nc.gpsimd.dma_start(gnb[:], gnb_d)
            # bv broadcast to all partitions (stride-0 partition DMA)
            bvrep = singles.tile([P, C], F32)
            bv_b = bass.AP(tensor=bv_d.tensor, offset=bv_d.offset,
                           ap=[[0, P], bv_d.ap[1]])
            nc.gpsimd.dma_start(out=bvrep[:], in_=bv_b)
            # xf (f32 residual copy) on the otherwise-idle gpsimd queue so the
            # sync/scalar rings stay clear for the latency-critical xbf
            for h in range(4):
                sl = slice(h * 1024, (h + 1) * 1024)
                nc.gpsimd.dma_start(xf0[:, sl], xf_d[0, :, sl])
                nc.gpsimd.dma_start(xf1[:, sl], xf_d[1, :, sl])
            epsap = singles.tile([P, 1], F32)
            nc.vector.memset(epsap[:], EPS)
            ebias = singles.tile([P, 1], F32)
            nc.vector.memset(ebias[:], EBIAS)

            # ---- PE warm-up: junk matmuls paced by the wq DMA then by the
            # x chunks, keeping HAM at K=8/8 until the projection burst ----
            wflat = wq[:, :, :].rearrange("p a b -> p (a b)")
            for i in range(8):
                warm = po.tile([P, SW], F32, tag="po", name=f"warmA{i}")
                nc.tensor.matmul(warm[:], wq[:, 0, 0:P], wflat,
                                 start=True, stop=True)

            # ---- GroupNorm.  bn_stats interleaved across the two halves in
            # DMA-arrival order; group aggregation via the amat matmul; one
            # batched Ln+Exp for rstd (avoids ACT table thrash); hs bf16 ----
            hs = hsp.tile([P, 2, N], BF16, tag="hs")
            # t0 per-channel stats on the DVE (bn_stats), t1 in parallel on
            # the otherwise-idle ACT via accum_out sums -- both halves'
            # stats finish right behind the DMA instead of serializing
            st0 = small.tile([P, 8, 6], F32, tag="gnst0")
            g1 = small.tile([P, 8], F32, tag="gns1")
            for h in range(4):
                hsl = slice(h * 1024, (h + 1) * 1024)
                xre = xbs[0][:, hsl].rearrange("p (s f) -> p s f", f=512)
                for sg in range(2):
                    nc.vector.bn_stats(st0[:, 2 * h + sg, :], xre[:, sg, :])
                sq = small.tile([P, 1024], BF16, tag="sq")
                nc.scalar.activation(sq[:], xbs[1][:, hsl], AF.Identity,
                                     accum_out=g1[:, h:h + 1])
                sq2 = small.tile([P, 1024], BF16, tag="sq")
                nc.scalar.activation(sq2[:], xbs[1][:, hsl], AF.Square,
                                     accum_out=g1[:, 4 + h:5 + h])
                for t in range(2):
                    # warm-up matmuls paced by chunk arrival
                    warm = po.tile([P, SW], F32, tag="po", name=f"wB{t}_{h}")
                    nc.tensor.matmul(warm[:], wq[:, 0, 0:P],
                                     xbs[t][:, h * 1024:h * 1024 + SW],
                                     start=True, stop=True)
                    warm2 = po.tile([P, SW], F32, tag="po", name=f"wC{t}_{h}")
                    nc.tensor.matmul(warm2[:], wq[:, 0, 0:P],
                                     xbs[t][:, h * 1024 + SW:(h + 1) * 1024],
                                     start=True, stop=True)
            # more warm-up, paced by the (slower) xf chunks: bridges the gap
            # between the bn_stats tail and the first projection matmuls
            for h in range(4):
                warm = po.tile([P, SW], F32, tag="po", name=f"wF{h}")
                nc.tensor.matmul(
                    warm[:], wq[:, 0, 0:P],
                    xf0[:, h * 1024:h * 1024 + 256].bitcast(BF16),
                    start=True, stop=True)
            gms = small.tile([P, 2], F32, tag="gm")   # group mean per half
            gvs = small.tile([P, 2], F32, tag="gv")   # group var per half
            for t in range(2):
                # stats2 = [mu, E[x^2]] per channel, f32r for the matmul
                stats2 = small.tile([P, 2], F32R, tag="gnst2")
                if t == 0:
                    mv = small.tile([P, 2], F32, tag="gnmv")
                    nc.vector.bn_aggr(mv[:], st0[:])
                    musq = small.tile([P, 1], F32, tag="gnmusq")
                    nc.vector.tensor_mul(musq[:], mv[:, 0:1], mv[:, 0:1])
                    nc.vector.tensor_copy(stats2[:, 0:1], mv[:, 0:1])
                    nc.vector.tensor_add(stats2[:, 1:2], mv[:, 1:2], musq[:])
                else:
                    pair = small.tile([P, 4], F32, tag="gnp")
                    nc.vector.tensor_add(pair[:, 0:2], g1[:, 0:2], g1[:, 2:4])
                    nc.vector.tensor_add(pair[:, 2:4], g1[:, 4:6], g1[:, 6:8])
                    tot = small.tile([P, 2], F32, tag="gnt")
                    nc.vector.tensor_add(tot[:, 0:1], pair[:, 0:1],
                                         pair[:, 1:2])
                    nc.vector.tensor_add(tot[:, 1:2], pair[:, 2:3],
                                         pair[:, 3:4])
                    nc.vector.tensor_scalar(stats2[:], tot[:], 1.0 / N, None,
                                            ALU.mult)
                # group-aggregate (mean over 8 channels) and broadcast back
                gp = ps.tile([P, 2, SW], F32, tag="ps", name=f"gn{t}")
                nc.tensor.matmul(gp[:, 0, 0:2], amat[:], stats2[:],
                                 start=True, stop=True)
                gs = small.tile([P, 2], F32, tag="gnagg")
                nc.vector.tensor_copy(gs[:], gp[:, 0, 0:2])
                gmusq = small.tile([P, 1], F32, tag="gnmusq2")
                nc.vector.tensor_mul(gmusq[:], gs[:, 0:1], gs[:, 0:1])
                nc.vector.tensor_copy(gms[:, t:t + 1], gs[:, 0:1])
                nc.vector.tensor_tensor(gvs[:, t:t + 1], gs[:, 1:2], gmusq[:],
                                        ALU.subtract)
            # rstd = exp(-0.5 * ln(var + eps)), both halves in one pass
            lnv = small.tile([P, 2], F32, tag="gnln")
            nc.scalar.activation(lnv[:], gvs[:], AF.Ln, bias=epsap[:],
                                 scale=1.0)
            rstds = small.tile([P, 2], F32, tag="gnrstd")
            nc.scalar.activation(rstds[:], lnv[:], AF.Exp, bias=0.0,
                                 scale=-0.5)
            for t in range(2):
                alpha = small.tile([P, 1], F32, tag="gnalpha")
                nc.vector.tensor_mul(alpha[:], rstds[:, t:t + 1],
                                     gnw[:, t:t + 1])
                atmp = small.tile([P, 1], F32, tag="gnatmp")
                nc.vector.tensor_mul(atmp[:], gms[:, t:t + 1], alpha[:])
                beta = small.tile([P, 1], F32, tag="gnbeta")
                nc.vector.tensor_tensor(beta[:], gnb[:, t:t + 1], atmp[:],
                                        ALU.subtract)
                for hh in range(2):
                    nc.vector.tensor_scalar(hs[:, t, hh * 2048:(hh + 1) * 2048],
                                            xbs[t][:, hh * 2048:(hh + 1) * 2048],
                                            alpha[:], beta[:], ALU.mult, ALU.add)
                if t == 0:
                    for hh in range(2):
                        warm = po.tile([P, SW], F32, tag="po", name=f"wD{hh}")
                        nc.tensor.matmul(warm[:], wq[:, 0, 0:P],
                                         hs[:, 0, hh * SW:(hh + 1) * SW],
                                         start=True, stop=True)

            # ---- q/k projections (bf16) -> fp8 qT/kT (bias-cast on ACT,
            # which is otherwise idle until the softmax phase) ----
            qT = qk.tile([P, 2, NQ], FP8, tag="qT")
            kT = qk.tile([P, 2, N], FP8, tag="kT")
            for (wt, bt, dst, nblk) in ((wq, bq, qT, NQ // SW),
                                        (wk, bk, kT, N // SW)):
                for ch in range(2):
                    for j in range(nblk // 2):
                        sp = ps.tile([P, 2, SW], F32, tag="ps")
                        for i in range(2):
                            b = 2 * j + i
                            for ko in range(2):
                                nc.tensor.matmul(
                                    sp[:, i, :],
                                    wt[:, ko, ch * P:(ch + 1) * P],
                                    hs[:, ko, b * SW:(b + 1) * SW],
                                    start=(ko == 0), stop=(ko == 1))
                        nc.scalar.activation(
                            dst[:, ch, 2 * j * SW:(2 * j + 2) * SW],
                            sp[:, 0:2, :].rearrange("p a b -> p (a b)"),
                            AF.Identity, bias=bt[:, ch:ch + 1], scale=1.0)

            # ---- v projection (bf16) -> fp8 row-major v; bias on gpsimd ----
            v = vpool.tile([P, MT, C], FP8)
            for m in range(MT):
                vp = po.tile([P, SW], F32, tag="po", name=f"vp{m}")
                for ko in range(2):
                    nc.tensor.matmul(vp[:, 0:C], hs[:, ko, m * P:(m + 1) * P],
                                     wv[:, ko, :], start=(ko == 0),
                                     stop=(ko == 1))
                nc.vector.tensor_add(v[:, m, :], vp[:, 0:C], bvrep[:])

            # ---- attention strips (fp8 DoubleRow) ----
            # zp is allocated once and zeroed: the Z chains only ever write
            # partitions 0/32/64/96, and the all-ones summing matmul must
            # not see stale garbage in the other rows.
            zp = pz.tile([P, SW], F32, tag="pz")
            nc.vector.memset(zp[:], 0.0)
            for s in range(NS):
                ns = slice(s * SW, (s + 1) * SW)
                es = espool.tile([P, MT, SW], FP8, tag="es")
                op0 = po.tile([P, SW], F32, tag="po", name=f"op{s}_0")
                op1 = po.tile([P, SW], F32, tag="po", name=f"op{s}_1")
                ops = (op0, op1)
                for jp in range(MP):
                    sp = ps.tile([P, 2, SW], F32, tag="ps")
                    for i in range(2):
                        m = 2 * jp + i
                        nc.tensor.matmul(
                            sp[:, i, :],
                            kT[:, :, m * P:(m + 1) * P],
                            qT[:, :, ns],
                            start=True, stop=True, perf_mode=DR)
                    nc.scalar.activation(es[:, 2 * jp:2 * jp + 2, :], sp[:],
                                         AF.Exp, bias=ebias[:], scale=ISCALE)
                    # attn @ v
                    for ch in range(2):
                        nc.tensor.matmul(
                            ops[ch],
                            v[:, 2 * jp:2 * jp + 2, ch * P:(ch + 1) * P],
                            es[:, 2 * jp:2 * jp + 2, :],
                            start=(jp == 0), stop=(jp == MP - 1),
                            perf_mode=DR)
                # softmax denominators: Z = ones.T @ P^T as one DoubleRow
                # M=1 chain (K=256 per matmul, so 16 instructions)
                for jp in range(MP):
                    nc.tensor.matmul(zp[0:1, :], ones8[:, :, 0:1],
                                     es[:, 2 * jp:2 * jp + 2, :],
                                     start=(jp == 0), stop=(jp == MP - 1),
                                     perf_mode=DR)
                # broadcast Z to all partitions in one matmul; then 1/Z via
                # the fast DVE approximation
                zsb = small.tile([P, SW], F32R, tag="zsb")
                nc.vector.tensor_copy(zsb[:], zp[:])
                zb = po.tile([P, SW], F32, tag="po", name=f"zb{s}")
                nc.tensor.matmul(zb[:], onesb[:], zsb[:],
                                 start=True, stop=True)
                zbs = small.tile([P, SW], F32, tag="zbs")
                nc.vector.tensor_copy(zbs[:], zb[:])
                rzall = small.tile([P, SW], F32, tag="rzall")
                nc.vector.reciprocal_approx_fast(rzall[:], zbs[:])
                o = opool.tile([P, 2, SW], BF16, tag="o")
                for ch in range(2):
                    nc.vector.tensor_mul(o[:, ch, :], ops[ch], rzall[:])
                # out projection (bf16) + bias + residual + 2^-0.5
                for ch in range(2):
                    op2 = po.tile([P, SW], F32, tag="po", name=f"op2_{s}_{ch}")
                    for ko in range(2):
                        nc.tensor.matmul(op2[:],
                                         wo[:, ko, ch * P:(ch + 1) * P],
                                         o[:, ko, :],
                                         start=(ko == 0), stop=(ko == 1))
                    z2 = zfpool.tile([P, SW], F32, tag="zf")
                    nc.vector.tensor_scalar(z2[:], op2[:], RS2,
                                            bos[:, ch:ch + 1],
                                            ALU.mult, ALU.add)
                    fin = finpool.tile([P, SW], F32, tag="fin")
                    nc.vector.scalar_tensor_tensor(
                        out=fin[:], in0=xfs[ch][:, ns], scalar=RS2,
                        in1=z2[:], op0=ALU.mult, op1=ALU.add)
                    nc.sync.dma_start(out_d[ch, :, ns], fin[:])

    nc.finalize()
    return nc


def _get_nc():
    if "nc" not in _prog_cache:
        _prog_cache["nc"] = _build_nc()
    return _prog_cache["nc"]


def _make_in_maps(x, gn_weight, gn_bias, Wq, bq, Wk, bk, Wv, bv, Wo, bo):
    x = np.asarray(x, dtype=np.float32)
    f32 = lambda a: np.ascontiguousarray(np.asarray(a, dtype=np.float32))
    BF = ml_dtypes.bfloat16

    def packT(b_vec):  # [256] -> [128, 2] (c_out_in, c_out_half)
        return np.ascontiguousarray(f32(b_vec).reshape(2, P).T)

    def w16(W):  # [C, C] -> [2, 128, C] bf16 of W.T
        return np.ascontiguousarray(
            np.asarray(W, np.float32).T.reshape(2, P, C).astype(BF))

    amat = np.zeros((P, P), np.float32)
    for g in range(P // GS):
        amat[g * GS:(g + 1) * GS, g * GS:(g + 1) * GS] = 1.0 / GS
    onesb = np.ones((P, P), np.float32)

    common = {
        "wq16": w16(Wq),
        "wk16": w16(Wk),
        "wv16": w16(Wv),
        "wo16": w16(Wo),
        "bqp": packT(bq),
        "bkp": packT(bk),
        "bosp": packT(np.asarray(bo, dtype=np.float32) * RS2),
        "bv": f32(bv).reshape(1, C),
        "gnw": packT(gn_weight),
        "gnb": packT(gn_bias),
        "amat": amat,
        "onesb": onesb,
        "ones8": np.ones((P, 32), ml_dtypes.float8_e4m3fn),
    }

    in_maps = []
    for core in range(8):
        b, half = core // 2, core % 2
        xt = x[b].reshape(C, N)
        if half:
            xt = np.roll(xt, -NQ, axis=1)
        xt = np.ascontiguousarray(xt).reshape(2, P, N)
        in_maps.append({"xbf": xt.astype(BF), "xf": xt, **common})
    return in_maps


def _assemble(results, B):
    out = np.empty((B, C, N), np.float32)
    for core in range(2 * B):
        b, half = core // 2, core % 2
        out[b, :, half * NQ:(half + 1) * NQ] = results[core]["out"].reshape(C, NQ)
    return out.reshape(B, C, 64, 64)


def kernel(x, gn_weight, gn_bias, Wq, bq, Wk, bk, Wv, bv, Wo, bo):
    x = np.asarray(x, dtype=np.float32)
    in_maps = _make_in_maps(x, gn_weight, gn_bias, Wq, bq, Wk, bk, Wv, bv, Wo, bo)
    nc = _get_nc()
    res = run_bass_kernel_spmd(nc, in_maps, list(range(8)))
    return _assemble(res.results, x.shape[0])


# revision 34
# speedup vs baseline: 1.2323x; 1.2323x over previous
"""AttnBlock (GroupNorm -> single-head self-attention -> residual) on 8 TRN2 cores.

Sharding: B=4 batch elements x 2 query-token halves = 8 cores (SPMD, no
collectives).  Each core receives the full (rolled) channel-major batch
element x^T [C=256, HW=4096], computes GroupNorm + k/v for all 4096
tokens, and q/scores/attention/out-proj for its 2048-token half.

Numerics plan (rel-err budget 2e-2; measured ~5e-3):
  x arrives twice: bf16 (GN stats/apply -- fast DMA) and f32 (residual).
  GN apply -> hs bf16.  q/k/v projections in bf16 (fp32 PSUM).
  qT/kT/v quantized to fp8e4m3; scores and attn@v run fp8 DoubleRow
  matmuls (K=256 per instruction via [128,2,*] APs).  exp(S/16 - 2)
  emitted as fp8 (the -2 cancels between numerator and denominator and
  keeps exp < 55 << 448 = e4m3 max).  Softmax division happens on the
  normalized o^T in bf16 before the bf16 out-projection.

Layout is channel-major throughout (tokens on the free axis):
  hs^T = GN(x^T)                      [C, N]   bf16
  q^T/k^T = W^T.T @ hs^T + b          [C, NQ/N] fp8
  v   = hs^T.T @ Wv^T + bv            [N, C]   fp8 (row-major)
  S^T = kT.T @ qT   (DoubleRow)       [N, SW]  per 512-wide query strip
  P^T = exp(S^T/16 - 2) -> fp8
  Z   = ones.T @ P^T (DoubleRow, M=1) [1, SW]
  o^T = (v.T @ P^T) * (1/Z)           [C, SW]  bf16
  out^T = Wo^T.T @ o^T; final = (x + out + bo) * 2^-0.5
"""

import numpy as np
import ml_dtypes

import concourse.bass as bass
import concourse.tile as tile
from concourse import bacc, mybir
from concourse.bass_utils import run_bass_kernel_spmd

dt = mybir.dt
F32, F32R, BF16, FP8 = dt.float32, dt.float32r, dt.bfloat16, dt.float8e4
AF = mybir.ActivationFunctionType
ALU = mybir.AluOpType
DR = mybir.MatmulPerfMode.DoubleRow

P = 128          # partitions
C = 256          # channels
N = 4096         # tokens per batch element (64*64)
NQ = 2048        # query tokens per core
SW = 512         # query strip width
NS = NQ // SW    # 4 strips
MT = N // P      # 32 key m-tiles
MP = MT // 2     # 16 key m-tile pairs (DoubleRow)
GS = 8           # channels per group (256 / 32 groups)
EPS = 1e-6
ISCALE = 1.0 / 16.0       # attention scale c**-0.5
EBIAS = -2.0              # exp range shift; cancels in softmax
RS2 = float(2.0 ** -0.5)  # output residual scale

_prog_cache = {}


def _build_nc():
    nc = bacc.Bacc("TRN2", target_bir_lowering=False, debug=False, num_devices=8)

    def inp(name, shape, d=F32):
        return nc.dram_tensor(name, shape, d, kind="ExternalInput").ap()

    xbf_d = inp("xbf", [2, P, N], BF16)    # [c_half, c_in, n] bf16 (GN path)
    xf_d = inp("xf", [2, P, N], F32)       # f32 copy (residual path)
    wq_d = inp("wq16", [2, P, C], BF16)    # [ko, ci_in, c_out] = Wq.T halves
    wk_d = inp("wk16", [2, P, C], BF16)
    wv_d = inp("wv16", [2, P, C], BF16)
    wo_d = inp("wo16", [2, P, C], BF16)
    bq_d = inp("bqp", [P, 2])              # [c_out_in, c_out_half]
    bk_d = inp("bkp", [P, 2])
    bos_d = inp("bosp", [P, 2])            # bo * 2^-0.5, packed
    bv_d = inp("bv", [1, C])
    gnw_d = inp("gnw", [P, 2])
    gnb_d = inp("gnb", [P, 2])
    amat_d = inp("amat", [P, P])           # block-diag 8x8 of 1/8
    onesb_d = inp("onesb", [P, P])         # all-ones (rows 0:4 used)
    ones8_d = inp("ones8", [P, 32], FP8)   # fp8 ones, 16B-strided pairs
    out_d = nc.dram_tensor("out", [2, P, NQ], F32, kind="ExternalOutput").ap()

    with tile.TileContext(nc) as tc:
        with (
            tc.tile_pool(name="singles", bufs=1) as singles,
            tc.tile_pool(name="xpool", bufs=1) as xpool,
            tc.tile_pool(name="hsp", bufs=1) as hsp,
            tc.tile_pool(name="qk", bufs=1) as qk,
            tc.tile_pool(name="vpool", bufs=1) as vpool,
            tc.tile_pool(name="espool", bufs=2) as espool,
            tc.tile_pool(name="opool", bufs=2) as opool,
            tc.tile_pool(name="zfpool", bufs=2) as zfpool,
            tc.tile_pool(name="finpool", bufs=2) as finpool,
            tc.tile_pool(name="small", bufs=2) as small,
            tc.tile_pool(name="ps", bufs=2, space="PSUM") as ps,      # 4 banks
            tc.tile_pool(name="pz", bufs=1, space="PSUM") as pz,      # 1 bank
            tc.tile_pool(name="po", bufs=3, space="PSUM") as po,      # 3 banks
        ):
            # ---- x loads: bf16 halves first (GN critical path), f32 after ----
            xb0 = xpool.tile([P, N], BF16, tag="xb0")
            xb1 = xpool.tile([P, N], BF16, tag="xb1")
            xf0 = xpool.tile([P, N], F32, tag="xf0")
            xf1 = xpool.tile([P, N], F32, tag="xf1")
            for h in range(4):
                sl = slice(h * 1024, (h + 1) * 1024)
                nc.sync.dma_start(xb0[:, sl], xbf_d[0, :, sl])
                nc.scalar.dma_start(xb1[:, sl], xbf_d[1, :, sl])
            xbs = (xb0, xb1)
            xfs = (xf0, xf1)

            # ---- constants / weights (gpsimd queue) ----
            wq = singles.tile([P, 2, C], BF16)
            wk = singles.tile([P, 2, C], BF16)
            wv = singles.tile([P, 2, C], BF16)
            wo = singles.tile([P, 2, C], BF16)
            for t_, d_ in ((wq, wq_d), (wk, wk_d), (wv, wv_d), (wo, wo_d)):
                for ko in range(2):
                    nc.gpsimd.dma_start(t_[:, ko, :], d_[ko])
            amat = singles.tile([P, P], F32R)
            nc.gpsimd.dma_start(amat[:], amat_d.bitcast(F32R))
            onesb = singles.tile([P, P], F32R)
            nc.gpsimd.dma_start(onesb[:], onesb_d.bitcast(F32R))
            ones8 = singles.tile([P, 2, 16], FP8)
            nc.gpsimd.dma_start(
                ones8[:].rearrange("p a b -> p (a b)"), ones8_d)
            bq = singles.tile([P, 2], F32)
            nc.gpsimd.dma_start(bq[:], bq_d)
            bk = singles.tile([P, 2], F32)
            nc.gpsimd.dma_start(bk[:], bk_d)
            bos = singles.tile([P, 2], F32)
            nc.gpsimd.dma_start(bos[:], bos_d)
            gnw = singles.tile([P, 2], F32)
            nc.gpsimd.dma_start(gnw[:], gnw_d)
            gnb = singles.tile([P, 2], F32)
            nc.gpsimd.dma_start(gnb[:], gnb_d)
            # bv broadcast to all partitions (stride-0 partition DMA)
            bvrep = singles.tile([P, C], F32)
            bv_b = bass.AP(tensor=bv_d.tensor, offset=bv_d.offset,
                           ap=[[0, P], bv_d.ap[1]])
            nc.gpsimd.dma_start(out=bvrep[:], in_=bv_b)
            # xf (f32 residual copy) on the otherwise-idle gpsimd queue so the
            # sync/scalar rings stay clear for the latency-critical xbf
            for h in range(4):
                sl = slice(h * 1024, (h + 1) * 1024)
                nc.gpsimd.dma_start(xf0[:, sl], xf_d[0, :, sl])
                nc.gpsimd.dma_start(xf1[:, sl], xf_d[1, :, sl])
            epsap = singles.tile([P, 1], F32)
            nc.vector.memset(epsap[:], EPS)
            ebias = singles.tile([P, 1], F32)
            nc.vector.memset(ebias[:], EBIAS)

            # ---- PE warm-up: junk matmuls paced by the wq DMA then by the
            # x chunks, keeping HAM at K=8/8 until the projection burst ----
            wflat = wq[:, :, :].rearrange("p a b -> p (a b)")
            for i in range(8):
                warm = po.tile([P, SW], F32, tag="po", name=f"warmA{i}")
                nc.tensor.matmul(warm[:], wq[:, 0, 0:P], wflat,
                                 start=True, stop=True)

            # ---- GroupNorm.  bn_stats interleaved across the two halves in
            # DMA-arrival order; group aggregation via the amat matmul; one
            # batched Ln+Exp for rstd (avoids ACT table thrash); hs bf16 ----
            hs = hsp.tile([P, 2, N], BF16, tag="hs")
            # t0 per-channel stats on the DVE (bn_stats), t1 in parallel on
            # the otherwise-idle ACT via accum_out sums -- both halves'
            # stats finish right behind the DMA instead of serializing
            st0 = small.tile([P, 8, 6], F32, tag="gnst0")
            g1 = small.tile([P, 8], F32, tag="gns1")
            for h in range(4):
                hsl = slice(h * 1024, (h + 1) * 1024)
                xre = xbs[0][:, hsl].rearrange("p (s f) -> p s f", f=512)
                for sg in range(2):
                    nc.vector.bn_stats(st0[:, 2 * h + sg, :], xre[:, sg, :])
                sq = small.tile([P, 1024], BF16, tag="sq")
                nc.scalar.activation(sq[:], xbs[1][:, hsl], AF.Identity,
                                     accum_out=g1[:, h:h + 1])
                sq2 = small.tile([P, 1024], BF16, tag="sq")
                nc.scalar.activation(sq2[:], xbs[1][:, hsl], AF.Square,
                                     accum_out=g1[:, 4 + h:5 + h])
                for t in range(2):
                    # warm-up matmuls paced by chunk arrival
                    warm = po.tile([P, SW], F32, tag="po", name=f"wB{t}_{h}")
                    nc.tensor.matmul(warm[:], wq[:, 0, 0:P],
                                     xbs[t][:, h * 1024:h * 1024 + SW],
                                     start=True, stop=True)
                    warm2 = po.tile([P, SW], F32, tag="po", name=f"wC{t}_{h}")
                    nc.tensor.matmul(warm2[:], wq[:, 0, 0:P],
                                     xbs[t][:, h * 1024 + SW:(h + 1) * 1024],
                                     start=True, stop=True)

            gms = small.tile([P, 2], F32, tag="gm")   # group mean per half
            gvs = small.tile([P, 2], F32, tag="gv")   # group var per half
            for t in range(2):
                # stats2 = [mu, E[x^2]] per channel, f32r for the matmul
                stats2 = small.tile([P, 2], F32R, tag="gnst2")
                if t == 0:
                    mv = small.tile([P, 2], F32, tag="gnmv")
                    nc.vector.bn_aggr(mv[:], st0[:])
                    musq = small.tile([P, 1], F32, tag="gnmusq")
                    nc.vector.tensor_mul(musq[:], mv[:, 0:1], mv[:, 0:1])
                    nc.vector.tensor_copy(stats2[:, 0:1], mv[:, 0:1])
                    nc.vector.tensor_add(stats2[:, 1:2], mv[:, 1:2], musq[:])
                else:
                    pair = small.tile([P, 4], F32, tag="gnp")
                    nc.vector.tensor_add(pair[:, 0:2], g1[:, 0:2], g1[:, 2:4])
                    nc.vector.tensor_add(pair[:, 2:4], g1[:, 4:6], g1[:, 6:8])
                    tot = small.tile([P, 2], F32, tag="gnt")
                    nc.vector.tensor_add(tot[:, 0:1], pair[:, 0:1],
                                         pair[:, 1:2])
                    nc.vector.tensor_add(tot[:, 1:2], pair[:, 2:3],
                                         pair[:, 3:4])
                    nc.vector.tensor_scalar(stats2[:], tot[:], 1.0 / N, None,
                                            ALU.mult)
                # group-aggregate (mean over 8 channels) and broadcast back
                gp = ps.tile([P, 2, SW], F32, tag="ps", name=f"gn{t}")
                nc.tensor.matmul(gp[:, 0, 0:2], amat[:], stats2[:],
                                 start=True, stop=True)
                gs = small.tile([P, 2], F32, tag="gnagg")
                nc.vector.tensor_copy(gs[:], gp[:, 0, 0:2])
                gmusq = small.tile([P, 1], F32, tag="gnmusq2")
                nc.vector.tensor_mul(gmusq[:], gs[:, 0:1], gs[:, 0:1])
                nc.vector.tensor_copy(gms[:, t:t + 1], gs[:, 0:1])
                nc.vector.tensor_tensor(gvs[:, t:t + 1], gs[:, 1:2], gmusq[:],
                                        ALU.subtract)
            # rstd = exp(-0.5 * ln(var + eps)), both halves in one pass
            lnv = small.tile([P, 2], F32, tag="gnln")
            nc.scalar.activation(lnv[:], gvs[:], AF.Ln, bias=epsap[:],
                                 scale=1.0)
            rstds = small.tile([P, 2], F32, tag="gnrstd")
            nc.scalar.activation(rstds[:], lnv[:], AF.Exp, bias=0.0,
                                 scale=-0.5)
            for t in range(2):
                alpha = small.tile([P, 1], F32, tag="gnalpha")
                nc.vector.tensor_mul(alpha[:], rstds[:, t:t + 1],
                                     gnw[:, t:t + 1])
                atmp = small.tile([P, 1], F32, tag="gnatmp")
                nc.vector.tensor_mul(atmp[:], gms[:, t:t + 1], alpha[:])
                beta = small.tile([P, 1], F32, tag="gnbeta")
                nc.vector.tensor_tensor(beta[:], gnb[:, t:t + 1], atmp[:],
                                        ALU.subtract)
                for hh in range(2):
                    nc.vector.tensor_scalar(hs[:, t, hh * 2048:(hh + 1) * 2048],
                                            xbs[t][:, hh * 2048:(hh + 1) * 2048],
                                            alpha[:], beta[:], ALU.mult, ALU.add)
                if t == 0:
                    for hh in range(2):
                        warm = po.tile([P, SW], F32, tag="po", name=f"wD{hh}")
                        nc.tensor.matmul(warm[:], wq[:, 0, 0:P],
                                         hs[:, 0, hh * SW:(hh + 1) * SW],
                                         start=True, stop=True)

            # ---- q/k projections (bf16) -> fp8 qT/kT (bias-cast on ACT,
            # which is otherwise idle until the softmax phase) ----
            qT = qk.tile([P, 2, NQ], FP8, tag="qT")
            kT = qk.tile([P, 2, N], FP8, tag="kT")
            for (wt, bt, dst, nblk) in ((wq, bq, qT, NQ // SW),
                                        (wk, bk, kT, N // SW)):
                for ch in range(2):
                    for j in range(nblk // 2):
                        sp = ps.tile([P, 2, SW], F32, tag="ps")
                        for i in range(2):
                            b = 2 * j + i
                            for ko in range(2):
                                nc.tensor.matmul(
                                    sp[:, i, :],
                                    wt[:, ko, ch * P:(ch + 1) * P],
                                    hs[:, ko, b * SW:(b + 1) * SW],
                                    start=(ko == 0), stop=(ko == 1))
                        nc.scalar.activation(
                            dst[:, ch, 2 * j * SW:(2 * j + 2) * SW],
                            sp[:, 0:2, :].rearrange("p a b -> p (a b)"),
                            AF.Identity, bias=bt[:, ch:ch + 1], scale=1.0)

            # ---- v projection (bf16) -> fp8 row-major v; bias on gpsimd ----
            v = vpool.tile([P, MT, C], FP8)
            for m in range(MT):
                vp = po.tile([P, SW], F32, tag="po", name=f"vp{m}")
                for ko in range(2):
                    nc.tensor.matmul(vp[:, 0:C], hs[:, ko, m * P:(m + 1) * P],
                                     wv[:, ko, :], start=(ko == 0),
                                     stop=(ko == 1))
                nc.vector.tensor_add(v[:, m, :], vp[:, 0:C], bvrep[:])

            # ---- attention strips (fp8 DoubleRow) ----
            # zp is allocated once and zeroed: the Z chains only ever write
            # partitions 0/32/64/96, and the all-ones summing matmul must
            # not see stale garbage in the other rows.
            zp = pz.tile([P, SW], F32, tag="pz")
            nc.vector.memset(zp[:], 0.0)
            for s in range(NS):
                ns = slice(s * SW, (s + 1) * SW)
                es = espool.tile([P, MT, SW], FP8, tag="es")
                op0 = po.tile([P, SW], F32, tag="po", name=f"op{s}_0")
                op1 = po.tile([P, SW], F32, tag="po", name=f"op{s}_1")
                ops = (op0, op1)
                for jp in range(MP):
                    sp = ps.tile([P, 2, SW], F32, tag="ps")
                    for i in range(2):
                        m = 2 * jp + i
                        nc.tensor.matmul(
                            sp[:, i, :],
                            kT[:, :, m * P:(m + 1) * P],
                            qT[:, :, ns],
                            start=True, stop=True, perf_mode=DR)
                    nc.scalar.activation(es[:, 2 * jp:2 * jp + 2, :], sp[:],
                                         AF.Exp, bias=ebias[:], scale=ISCALE)
                    # attn @ v
                    for ch in range(2):
                        nc.tensor.matmul(
                            ops[ch],
                            v[:, 2 * jp:2 * jp + 2, ch * P:(ch + 1) * P],
                            es[:, 2 * jp:2 * jp + 2, :],
                            start=(jp == 0), stop=(jp == MP - 1),
                            perf_mode=DR)
                # softmax denominators: Z = ones.T @ P^T as one DoubleRow
                # M=1 chain (K=256 per matmul, so 16 instructions)
                for jp in range(MP):
                    nc.tensor.matmul(zp[0:1, :], ones8[:, :, 0:1],
                                     es[:, 2 * jp:2 * jp + 2, :],
                                     start=(jp == 0), stop=(jp == MP - 1),
                                     perf_mode=DR)
                # broadcast Z to all partitions in one matmul; then 1/Z via
                # the fast DVE approximation
                zsb = small.tile([P, SW], F32R, tag="zsb")
                nc.vector.tensor_copy(zsb[:], zp[:])
                zb = po.tile([P, SW], F32, tag="po", name=f"zb{s}")
                nc.tensor.matmul(zb[:], onesb[:], zsb[:],
                                 start=True, stop=True)
                zbs = small.tile([P, SW], F32, tag="zbs")
                nc.vector.tensor_copy(zbs[:], zb[:])
                rzall = small.tile([P, SW], F32, tag="rzall")
                nc.vector.reciprocal_approx_fast(rzall[:], zbs[:])
                o = opool.tile([P, 2, SW], BF16, tag="o")
                for ch in range(2):
                    nc.vector.tensor_mul(o[:, ch, :], ops[ch], rzall[:])
                # out projection (bf16) + bias + residual + 2^-0.5
                for ch in range(2):
                    op2 = po.tile([P, SW], F32, tag="po", name=f"op2_{s}_{ch}")
                    for ko in range(2):
                        nc.tensor.matmul(op2[:],
                                         wo[:, ko, ch * P:(ch + 1) * P],
                                         o[:, ko, :],
                                         start=(ko == 0), stop=(ko == 1))
                    z2 = zfpool.tile([P, SW], F32, tag="zf")
                    nc.vector.tensor_scalar(z2[:], op2[:], RS2,
                                            bos[:, ch:ch + 1],
                                            ALU.mult, ALU.add)
                    fin = finpool.tile([P, SW], F32, tag="fin")
                    nc.vector.scalar_tensor_tensor(
                        out=fin[:], in0=xfs[ch][:, ns], scalar=RS2,
                        in1=z2[:], op0=ALU.mult, op1=ALU.add)
                    nc.sync.dma_start(out_d[ch, :, ns], fin[:])

    nc.finalize()
    return nc


def _get_nc():
    if "nc" not in _prog_cache:
        _prog_cache["nc"] = _build_nc()
    return _prog_cache["nc"]


def _make_in_maps(x, gn_weight, gn_bias, Wq, bq, Wk, bk, Wv, bv, Wo, bo):
    x = np.asarray(x, dtype=np.float32)
    f32 = lambda a: np.ascontiguousarray(np.asarray(a, dtype=np.float32))
    BF = ml_dtypes.bfloat16

    def packT(b_vec):  # [256] -> [128, 2] (c_out_in, c_out_half)
        return np.ascontiguousarray(f32(b_vec).reshape(2, P).T)

    def w16(W):  # [C, C] -> [2, 128, C] bf16 of W.T
        return np.ascontiguousarray(
            np.asarray(W, np.float32).T.reshape(2, P, C).astype(BF))

    amat = np.zeros((P, P), np.float32)
    for g in range(P // GS):
        amat[g * GS:(g + 1) * GS, g * GS:(g + 1) * GS] = 1.0 / GS
    onesb = np.ones((P, P), np.float32)

    common = {
        "wq16": w16(Wq),
        "wk16": w16(Wk),
        "wv16": w16(Wv),
        "wo16": w16(Wo),
        "bqp": packT(bq),
        "bkp": packT(bk),
        "bosp": packT(np.asarray(bo, dtype=np.float32) * RS2),
        "bv": f32(bv).reshape(1, C),
        "gnw": packT(gn_weight),
        "gnb": packT(gn_bias),
        "amat": amat,
        "onesb": onesb,
        "ones8": np.ones((P, 32), ml_dtypes.float8_e4m3fn),
    }

    in_maps = []
    for core in range(8):
        b, half = core // 2, core % 2
        xt = x[b].reshape(C, N)
        if half:
            xt = np.roll(xt, -NQ, axis=1)
        xt = np.ascontiguousarray(xt).reshape(2, P, N)
        in_maps.append({"xbf": xt.astype(BF), "xf": xt, **common})
    return in_maps


def _assemble(results, B):
    out = np.empty((B, C, N), np.float32)
    for core in range(2 * B):
        b, half = core // 2, core % 2
        out[b, :, half * NQ:(half + 1) * NQ] = results[core]["out"].reshape(C, NQ)
    return out.reshape(B, C, 64, 64)


def kernel(x, gn_weight, gn_bias, Wq, bq, Wk, bk, Wv, bv, Wo, bo):
    x = np.asarray(x, dtype=np.float32)
    in_maps = _make_in_maps(x, gn_weight, gn_bias, Wq, bq, Wk, bk, Wv, bv, Wo, bo)
    nc = _get_nc()
    res = run_bass_kernel_spmd(nc, in_maps, list(range(8)))
    return _assemble(res.results, x.shape[0])
